# revision 7
# baseline (speedup 1.0000x reference)
"""Trainium2 Bass kernel for a pre-LN transformer block (B=4, S=2048, H=12, D=64).

Sharding: 8 cores; core c -> batch b = c//2, parity p = c%2.
Each core handles 1024 query rows of its batch: local query block j (128 rows,
j=0..7) maps to global block g = 2j + p (stride-2 interleave balances the causal
load so every core runs an identical SPMD program).

Per core on-device pipeline (activations kept feature-major [feature, token]):
  LN1 over full 2048 tokens (for K/V) and over the 1024 query tokens (for Q),
  QKV projections, causal attention (scores computed transposed [keys, q] with
  key-tile-outer loop; softmax without max-subtraction; per-tile multiplicative
  masks from host), Wo + residual, LN2, MLP (exact GELU) + residual.

All matmuls in bf16 with fp32 PSUM accumulation; LN stats, residuals and
softmax normalization in fp32.
"""

import numpy as np

N_CORES = 8
B, S, H, D = 4, 2048, 12, 64
HID = 768
QL = 1024          # query rows per core
KT = HID // 128    # 6 feature k-tiles
TT = S // 128      # 16 token tiles
MH = 4 * HID // 128  # 24 tiles of the MLP hidden dim
EPS = 1e-5

_CACHE = {}


def _build_program():
    from contextlib import ExitStack
    import concourse.bass as bass
    import concourse.tile as tile
    from concourse import bacc, mybir

    F32 = mybir.dt.float32
    BF16 = mybir.dt.bfloat16
    Alu = mybir.AluOpType
    Act = mybir.ActivationFunctionType

    nc = bacc.Bacc("TRN2", target_bir_lowering=False, debug=False,
                   enable_asserts=False, num_devices=N_CORES)

    def din(name, shape, dt):
        return nc.dram_tensor(name, shape, dt, kind="ExternalInput").ap()

    # --- per-core inputs ---
    xbT = din("xbT", [HID, S], BF16)          # x[b].T, bf16
    xbTq = din("xbTq", [HID, QL], BF16)       # gathered query cols, bf16
    xTq = din("xTq", [HID, QL], F32)          # gathered query cols, f32 (residual)
    masks = din("masks", [TT, 128, 128], BF16)
    Wq = din("Wq", [HID, HID], BF16)
    Wk = din("Wk", [HID, HID], BF16)
    Wv = din("Wv", [HID, HID], BF16)
    Wo = din("Wo", [HID, HID], BF16)
    W1 = din("W1", [HID, 4 * HID], BF16)
    W2 = din("W2", [4 * HID, HID], BF16)
    ln1w = din("ln1w", [HID], F32)
    ln1b = din("ln1b", [HID], F32)
    ln2w = din("ln2w", [HID], F32)
    ln2b = din("ln2b", [HID], F32)
    bqs = din("bqs", [HID], F32)              # bq / sqrt(D)
    bk = din("bk", [HID], F32)
    bv = din("bv", [HID], F32)
    bo = din("bo", [HID], F32)
    b1 = din("b1", [4 * HID], F32)
    b2 = din("b2", [HID], F32)

    y = nc.dram_tensor("y", [HID, QL], F32, kind="ExternalOutput").ap()

    # --- DRAM scratch ---
    st_dram = nc.dram_tensor("st_scratch", [16, S], F32).ap()
    g_dram = nc.dram_tensor("g_scratch", [MH, 128, QL], BF16).ap()
    rec_dram = nc.dram_tensor("rec_scratch", [H, QL], F32).ap()

    def bcast(src_elem_ap, parts, n):
        """AP reading n consecutive DRAM elems broadcast across `parts` partitions."""
        return bass.AP(tensor=src_elem_ap.tensor, offset=src_elem_ap.offset,
                       ap=[[0, parts], [1, n]])

    with tile.TileContext(nc) as tc, ExitStack() as ctx:
        sb = ctx.enter_context(tc.tile_pool(name="sb", bufs=1))
        ps = ctx.enter_context(tc.tile_pool(name="ps", bufs=1, space="PSUM"))

        # ---------- constant / parameter tiles ----------
        ones_bf = sb.tile([128, 1], BF16, tag="ones")
        nc.vector.memset(ones_bf, 1.0)
        # packed params: [128, 80] f32
        # cols 0:6 ln1w, 6:12 ln1b, 12:18 ln2w, 18:24 ln2b, 24:30 bqs, 30:36 bk,
        # 36:42 bo, 42:48 b2, 48:72 b1, 72 eps
        par = sb.tile([128, 80], F32, tag="par")

        def load_cols(dst0, src, n):  # [n*128] -> [128, n] cols of par
            nc.scalar.dma_start(
                out=par[:, dst0:dst0 + n],
                in_=bass.AP(tensor=src.tensor, offset=src.offset,
                            ap=[[1, 128], [128, n]]))

        load_cols(0, ln1w, KT)
        load_cols(6, ln1b, KT)
        load_cols(12, ln2w, KT)
        load_cols(18, ln2b, KT)
        load_cols(24, bqs, KT)
        load_cols(30, bk, KT)
        load_cols(36, bo, KT)
        load_cols(42, b2, KT)
        load_cols(48, b1, MH)
        nc.vector.memset(par[:, 72:73], EPS)
        eps_t = par[:, 72:73]
        bv_b = sb.tile([128, HID], F32, tag="bv_b")
        nc.scalar.dma_start(out=bv_b, in_=bcast(bv[0], 128, HID))

        masks_sb = sb.tile([128, TT, 128], BF16, tag="masks")
        nc.scalar.dma_start(out=masks_sb, in_=masks.rearrange("t p q -> p t q"))

        # ---------- LN helper (feature-major), chunk-pipelined ----------
        def emit_ln(N, x_bf, w0, b0, out_bf, row0):
            nchunks = N // 512
            for c in range(nchunks):
                off = 512 * c
                cs = slice(off, off + 512)
                s_ps = ps.tile([1, 512], F32, tag="s", bufs=4, name=f"sps{row0}_{c}")
                q_ps = ps.tile([1, 512], F32, tag="s", bufs=4, name=f"qps{row0}_{c}")
                for kt in range(KT):
                    nc.tensor.matmul(s_ps, ones_bf, x_bf[:, kt, cs],
                                     start=(kt == 0), stop=(kt == KT - 1))
                for kt in range(KT):
                    sqc = sb.tile([128, 512], BF16, tag="sq", bufs=2,
                                  name=f"sq{row0}_{c}_{kt}")
                    nc.vector.tensor_mul(sqc, x_bf[:, kt, cs], x_bf[:, kt, cs])
                    nc.tensor.matmul(q_ps, ones_bf, sqc,
                                     start=(kt == 0), stop=(kt == KT - 1))
                s_sb = sb.tile([1, 512], F32, tag="sts", bufs=2, name=f"ssb{row0}_{c}")
                q_sb = sb.tile([1, 512], F32, tag="sts", bufs=2, name=f"qsb{row0}_{c}")
                nc.scalar.copy(s_sb, s_ps)
                nc.scalar.copy(q_sb, q_ps)
                nc.scalar.dma_start(out=st_dram[row0, off:off + 512], in_=s_sb)
                nc.scalar.dma_start(out=st_dram[row0 + 1, off:off + 512], in_=q_sb)

                def resh(row):
                    base = st_dram[row, off]
                    return bass.AP(tensor=base.tensor, offset=base.offset,
                                   ap=[[1, 128], [128, 4]])
                s_r = sb.tile([128, 4], F32, tag="str", bufs=8, name=f"sr{row0}_{c}")
                q_r = sb.tile([128, 4], F32, tag="str", bufs=8, name=f"qr{row0}_{c}")
                nc.scalar.dma_start(out=s_r, in_=resh(row0))
                nc.scalar.dma_start(out=q_r, in_=resh(row0 + 1))
                mu = sb.tile([128, 4], F32, tag="str", bufs=8, name=f"mu{row0}_{c}")
                e2 = sb.tile([128, 4], F32, tag="str", bufs=8, name=f"e2{row0}_{c}")
                nc.scalar.mul(mu, s_r, 1.0 / HID)
                nc.scalar.mul(e2, q_r, 1.0 / HID)
                var = sb.tile([128, 4], F32, tag="str", bufs=8, name=f"var{row0}_{c}")
                nc.vector.tensor_mul(var, mu, mu)
                nc.vector.tensor_sub(var, e2, var)
                sd = sb.tile([128, 4], F32, tag="str", bufs=8, name=f"sd{row0}_{c}")
                nc.scalar.activation(sd, var, Act.Sqrt, bias=eps_t, scale=1.0)
                a_r = sb.tile([128, 4], F32, tag="str", bufs=8, name=f"ar{row0}_{c}")
                nc.vector.reciprocal(a_r, sd)
                c_r = sb.tile([128, 4], F32, tag="str", bufs=8, name=f"cr{row0}_{c}")
                nc.vector.tensor_mul(c_r, mu, a_r)
                nc.scalar.mul(c_r, c_r, -1.0)
                nc.scalar.dma_start(out=resh(row0 + 2), in_=a_r)
                nc.scalar.dma_start(out=resh(row0 + 3), in_=c_r)
                a_b = sb.tile([128, 512], F32, tag="ab", bufs=2, name=f"ab{row0}_{c}")
                c_b = sb.tile([128, 512], F32, tag="ab", bufs=2, name=f"cb{row0}_{c}")
                nc.scalar.dma_start(out=a_b, in_=bcast(st_dram[row0 + 2, off], 128, 512))
                nc.scalar.dma_start(out=c_b, in_=bcast(st_dram[row0 + 3, off], 128, 512))
                for kt in range(KT):
                    t0 = sb.tile([128, 512], F32, tag="t0", bufs=2,
                                 name=f"t0{row0}_{c}_{kt}")
                    nc.vector.tensor_mul(t0, x_bf[:, kt, cs], a_b)
                    nc.vector.tensor_add(t0, t0, c_b)
                    nc.vector.tensor_scalar(out_bf[:, kt, cs], t0,
                                            par[:, w0 + kt:w0 + kt + 1],
                                            par[:, b0 + kt:b0 + kt + 1],
                                            Alu.mult, Alu.add)

        # ---------- Phase 1: LN1 (full) and LN1q ----------
        xbT_sb = sb.tile([128, KT, S], BF16, tag="fatA")
        for c in range(S // 512):
            nc.sync.dma_start(
                out=xbT_sb[:, :, 512 * c:512 * c + 512],
                in_=xbT.rearrange("(k p) s -> p k s", p=128)[:, :, 512 * c:512 * c + 512])
        xbTq_sb = sb.tile([128, KT, QL], BF16, tag="medB")
        for c in range(QL // 512):
            nc.sync.dma_start(
                out=xbTq_sb[:, :, 512 * c:512 * c + 512],
                in_=xbTq.rearrange("(k p) s -> p k s", p=128)[:, :, 512 * c:512 * c + 512])

        ln_bf = sb.tile([128, KT, S], BF16, tag="fatB")
        lnq_bf = sb.tile([128, KT, QL], BF16, tag="medA")
        emit_ln(S, xbT_sb, 0, 6, ln_bf, 0)
        emit_ln(QL, xbTq_sb, 0, 6, lnq_bf, 4)

        # ---------- Phase 2: QKV projections ----------
        K_sb = sb.tile([128, KT, S], BF16, tag="fatC")
        for n in range(S // 512):
            cs = slice(512 * n, 512 * n + 512)
            for mo in range(KT):
                wkt = sb.tile([128, KT, 128], BF16, tag="wk6", bufs=3,
                              name=f"wk{n}_{mo}")
                nc.sync.dma_start(
                    out=wkt,
                    in_=Wk.rearrange("(k p) m -> p k m", p=128)[:, :, 128 * mo:128 * mo + 128])
                pst = ps.tile([128, 512], F32, tag="s", bufs=4, name=f"kps{mo}_{n}")
                for kt in range(KT):
                    nc.tensor.matmul(pst, wkt[:, kt, :], ln_bf[:, kt, cs],
                                     start=(kt == 0), stop=(kt == KT - 1))
                nc.vector.tensor_scalar(K_sb[:, mo, cs], pst,
                                        par[:, 30 + mo:31 + mo], None, Alu.add)

        Q_sb = sb.tile([128, KT, QL], BF16, tag="qsb")
        for n in range(QL // 512):
            cs = slice(512 * n, 512 * n + 512)
            for mo in range(KT):
                wqt = sb.tile([128, KT, 128], BF16, tag="wk6", bufs=3,
                              name=f"wq{n}_{mo}")
                nc.sync.dma_start(
                    out=wqt,
                    in_=Wq.rearrange("(k p) m -> p k m", p=128)[:, :, 128 * mo:128 * mo + 128])
                pst = ps.tile([128, 512], F32, tag="s", bufs=4, name=f"qps2{mo}_{n}")
                for kt in range(KT):
                    nc.tensor.matmul(pst, wqt[:, kt, :], lnq_bf[:, kt, cs],
                                     start=(kt == 0), stop=(kt == KT - 1))
                # (q + bq)/sqrt(D): psum*0.125 + bqs (bqs pre-scaled on host)
                nc.vector.tensor_scalar(Q_sb[:, mo, cs], pst,
                                        0.125, par[:, 24 + mo:25 + mo],
                                        Alu.mult, Alu.add)

        # V: token-major [128, tt, 12*65], head h at cols 65h..65h+63, ones at 65h+64
        V_sb = sb.tile([128, TT, H * 65], BF16, tag="vsb")
        for fc in range(2):  # fout chunks of 384 = 6 heads
            wvt = sb.tile([128, KT, 384], BF16, tag="wv", bufs=1, name=f"wv{fc}")
            nc.sync.dma_start(
                out=wvt,
                in_=Wv.rearrange("(k p) m -> p k m", p=128)[:, :, 384 * fc:384 * fc + 384])
            for tt in range(TT):
                pst = ps.tile([128, 384], F32, tag="s", bufs=4, name=f"vps{tt}_{fc}")
                for kt in range(KT):
                    nc.tensor.matmul(pst, ln_bf[:, kt, 128 * tt:128 * tt + 128],
                                     wvt[:, kt, :],
                                     start=(kt == 0), stop=(kt == KT - 1))
                vdst = V_sb[:, tt, :].rearrange("p (h e) -> p h e", e=65)[:, 6 * fc:6 * fc + 6, 0:64]
                nc.vector.tensor_tensor(vdst, pst.rearrange("p (h d) -> p h d", d=64),
                                        bv_b[:, 384 * fc:384 * fc + 384].rearrange(
                                            "p (h d) -> p h d", d=64),
                                        Alu.add)
        for tt in range(TT):
            nc.vector.memset(
                V_sb[:, tt, :].rearrange("p (h e) -> p h e", e=65)[:, :, 64:65], 1.0)

        # ---------- Phase 3: attention ----------
        attn_bf = sb.tile([128, KT, QL], BF16, tag="medA")
        for h in range(H):
            kt = h // 2
            p0 = 64 * (h % 2)
            rows = slice(p0, p0 + 64)
            O_ps = ps.tile([65, QL], F32, tag="acc", bufs=2, name=f"ops{h}")
            prev = None  # (expS tile, q0, t)
            for t in range(TT):
                q0 = 128 * (t // 2)
                span = QL - q0
                expS = sb.tile([128, span], BF16, tag="expS", bufs=3,
                               name=f"es{h}_{t}")
                chunks = ([(q0, 512), (512, QL)] if q0 < 512 else [(q0, QL)])
                for (cs, ce) in chunks:
                    S_ps = ps.tile([128, ce - cs], F32, tag="s", bufs=4,
                                   name=f"scr{h}_{t}_{cs}")
                    nc.tensor.matmul(S_ps, K_sb[rows, kt, 128 * t:128 * t + 128],
                                     Q_sb[rows, kt, cs:ce], start=True, stop=True)
                    nc.scalar.activation(expS[:, cs - q0:ce - q0], S_ps, Act.Exp)
                nc.vector.tensor_mul(expS[:, 0:128], expS[:, 0:128],
                                     masks_sb[:, t, :])
                if prev is not None:
                    pe, pq0, pt = prev
                    for (cs, ce) in ([(pq0, 512), (512, QL)] if pq0 < 512
                                     else [(pq0, QL)]):
                        nc.tensor.matmul(O_ps[:, cs:ce],
                                         V_sb[:, pt, 65 * h:65 * h + 65],
                                         pe[:, cs - pq0:ce - pq0],
                                         start=(pt == 0), stop=False)
                prev = (expS, q0, t)
            pe, pq0, pt = prev
            nc.tensor.matmul(O_ps[:, pq0:QL], V_sb[:, pt, 65 * h:65 * h + 65],
                             pe, start=False, stop=True)
            rec = sb.tile([65, QL], F32, tag="rec", bufs=1, name=f"rec{h}")
            nc.vector.reciprocal(rec[64:65, :], O_ps[64:65, :])
            nc.scalar.dma_start(out=rec_dram[h, :], in_=rec[64:65, :])
            nc.scalar.dma_start(out=rec[0:64, :], in_=bcast(rec_dram[h, 0], 64, QL))
            if h % 2 == 0:
                nc.vector.tensor_mul(attn_bf[0:64, kt, :], O_ps[0:64, :], rec[0:64, :])
            else:
                stg = sb.tile([64, QL], BF16, tag="stg", bufs=1, name=f"stg{h}")
                nc.vector.tensor_mul(stg, O_ps[0:64, :], rec[0:64, :])
                nc.sync.dma_start(out=attn_bf[64:128, kt, :], in_=stg)

        # ---------- Phase 4: Wo + residual + LN2 ----------
        xTq_sb = sb.tile([128, KT, QL], F32, tag="fatA")
        nc.sync.dma_start(out=xTq_sb, in_=xTq.rearrange("(k p) s -> p k s", p=128))
        r_sb = sb.tile([128, KT, QL], F32, tag="fatB")
        rb_sb = sb.tile([128, KT, QL], BF16, tag="medB")
        for n in range(QL // 512):
            cs = slice(512 * n, 512 * n + 512)
            for mo in range(KT):
                wot = sb.tile([128, KT, 128], BF16, tag="wk6", bufs=3,
                              name=f"wo{n}_{mo}")
                nc.sync.dma_start(
                    out=wot,
                    in_=Wo.rearrange("(k p) m -> p k m", p=128)[:, :, 128 * mo:128 * mo + 128])
                pst = ps.tile([128, 512], F32, tag="s", bufs=4, name=f"ops2{mo}_{n}")
                for kt in range(KT):
                    nc.tensor.matmul(pst, wot[:, kt, :], attn_bf[:, kt, cs],
                                     start=(kt == 0), stop=(kt == KT - 1))
                nc.vector.scalar_tensor_tensor(r_sb[:, mo, cs], pst,
                                               par[:, 36 + mo:37 + mo],
                                               xTq_sb[:, mo, cs],
                                               Alu.add, Alu.add)
                nc.vector.tensor_copy(rb_sb[:, mo, cs], r_sb[:, mo, cs])
        ln2_bf = sb.tile([128, KT, QL], BF16, tag="medC")
        emit_ln(QL, rb_sb, 12, 18, ln2_bf, 8)

        # ---------- Phase 5: MLP ----------
        for mo in range(MH):
            w1t = sb.tile([128, KT, 128], BF16, tag="wk6", bufs=3, name=f"w1{mo}")
            nc.sync.dma_start(
                out=w1t,
                in_=W1.rearrange("(k p) m -> p k m", p=128)[:, :, 128 * mo:128 * mo + 128])
            gst = sb.tile([128, QL], BF16, tag="gst", bufs=2, name=f"gst{mo}")
            for n in range(QL // 512):
                cs = slice(512 * n, 512 * n + 512)
                pst = ps.tile([128, 512], F32, tag="s", bufs=4, name=f"h1ps{mo}_{n}")
                for kt in range(KT):
                    nc.tensor.matmul(pst, w1t[:, kt, :], ln2_bf[:, kt, cs],
                                     start=(kt == 0), stop=(kt == KT - 1))
                nc.scalar.activation(gst[:, cs], pst, Act.Gelu,
                                     bias=par[:, 48 + mo:49 + mo], scale=1.0)
            nc.sync.dma_start(out=g_dram[mo], in_=gst)

        y_sb = sb.tile([128, KT, QL], F32, tag="fatC")
        for n in range(QL // 512):
            cs = slice(512 * n, 512 * n + 512)
            psts = [ps.tile([128, 512], F32, tag=("s" if i < 4 else "acc"),
                            bufs=(4 if i < 4 else 2), name=f"yps{n}_{i}")
                    for i in range(KT)]
            for kp in range(MH // 2):
                w2t = sb.tile([128, 2, HID], BF16, tag="w2", bufs=2, name=f"w2{n}_{kp}")
                nc.sync.dma_start(
                    out=w2t,
                    in_=W2.rearrange("(a p) m -> p a m", p=128)[:, 2 * kp:2 * kp + 2, :])
                gk = sb.tile([128, 2, 512], BF16, tag="gk", bufs=2, name=f"gk{n}_{kp}")
                nc.sync.dma_start(
                    out=gk, in_=g_dram[2 * kp:2 * kp + 2, :, cs].rearrange(
                        "g p c -> p g c"))
                for j in range(2):
                    k2 = 2 * kp + j
                    for mo in range(KT):
                        nc.tensor.matmul(psts[mo], w2t[:, j, 128 * mo:128 * mo + 128],
                                         gk[:, j, :],
                                         start=(k2 == 0), stop=(k2 == MH - 1))
            for mo in range(KT):
                nc.vector.scalar_tensor_tensor(y_sb[:, mo, cs], psts[mo],
                                               par[:, 42 + mo:43 + mo],
                                               r_sb[:, mo, cs],
                                               Alu.add, Alu.add)
                nc.sync.dma_start(out=y[128 * mo:128 * mo + 128, cs],
                                  in_=y_sb[:, mo, cs])

    nc.compile()
    return nc


def _get_program():
    if "nc" not in _CACHE:
        _CACHE["nc"] = _build_program()
    return _CACHE["nc"]


def _prep_in_maps(inputs):
    import ml_dtypes
    bf = ml_dtypes.bfloat16
    f32 = np.float32

    x = np.ascontiguousarray(np.asarray(inputs["x"], dtype=f32))
    shared = {
        "Wq": np.ascontiguousarray(np.asarray(inputs["Wq"], f32).astype(bf)),
        "Wk": np.ascontiguousarray(np.asarray(inputs["Wk"], f32).astype(bf)),
        "Wv": np.ascontiguousarray(np.asarray(inputs["Wv"], f32).astype(bf)),
        "Wo": np.ascontiguousarray(np.asarray(inputs["Wo"], f32).astype(bf)),
        "W1": np.ascontiguousarray(np.asarray(inputs["W1"], f32).astype(bf)),
        "W2": np.ascontiguousarray(np.asarray(inputs["W2"], f32).astype(bf)),
        "ln1w": np.asarray(inputs["ln1_w"], f32),
        "ln1b": np.asarray(inputs["ln1_b"], f32),
        "ln2w": np.asarray(inputs["ln2_w"], f32),
        "ln2b": np.asarray(inputs["ln2_b"], f32),
        "bqs": (np.asarray(inputs["bq"], f32) / np.float32(np.sqrt(D))),
        "bk": np.asarray(inputs["bk"], f32),
        "bv": np.asarray(inputs["bv"], f32),
        "bo": np.asarray(inputs["bo"], f32),
        "b1": np.asarray(inputs["b1"], f32),
        "b2": np.asarray(inputs["b2"], f32),
    }

    in_maps = []
    qcols_all = []
    for c in range(N_CORES):
        b, p = c // 2, c % 2
        xT = np.ascontiguousarray(x[b].T)                        # [768, 2048]
        qcols = np.concatenate(
            [np.arange(128 * (2 * j + p), 128 * (2 * j + p) + 128) for j in range(8)])
        qcols_all.append(qcols)
        xTq = np.ascontiguousarray(xT[:, qcols])
        m = np.zeros((TT, 128, 128), np.float32)
        for t in range(TT):
            g = 2 * (t // 2) + p
            kk = 128 * t + np.arange(128)[:, None]
            qq = 128 * g + np.arange(128)[None, :]
            m[t] = (kk <= qq).astype(np.float32)
        im = dict(shared)
        im["xbT"] = xT.astype(bf)
        im["xbTq"] = xTq.astype(bf)
        im["xTq"] = xTq
        im["masks"] = m.astype(bf)
        in_maps.append(im)
    return in_maps, qcols_all


def kernel(**inputs):
    import sys, types
    # register the NTFF profile hook shim (harmless if profiling unused)
    if "antenv.axon_hooks" not in sys.modules:
        try:
            sys.path.insert(0, "/root/.axon_site")
            from trn_agent_boot.trn_boot import _ntff_profile_via_ctypes
            hook = _ntff_profile_via_ctypes("/opt/axon/libaxon_pjrt.so")
            mod = types.ModuleType("antenv.axon_hooks")
            mod.get_axon_ntff_profile_hook = lambda: hook
            mod.set_axon_ntff_profile_hook = lambda h: None
            import antenv  # noqa: F401
            sys.modules["antenv.axon_hooks"] = mod
        except Exception:
            pass

    from concourse.bass_utils import run_bass_kernel_spmd

    nc = _get_program()
    in_maps, qcols_all = _prep_in_maps(inputs)
    res = run_bass_kernel_spmd(nc, in_maps, core_ids=list(range(N_CORES)))
    out = np.zeros((B, S, HID), np.float32)
    for c in range(N_CORES):
        out[c // 2, qcols_all[c], :] = res.results[c]["y"].T
    return out


# revision 8
# speedup vs baseline: 1.0767x; 1.0767x over previous
"""Trainium2 Bass kernel for a pre-LN transformer block (B=4, S=2048, H=12, D=64).

Sharding: 8 cores; core c -> batch b = c//2, parity p = c%2.
Each core handles 1024 query rows of its batch: local query block j (128 rows,
j=0..7) maps to global block g = 2j + p (stride-2 interleave balances the causal
load so every core runs an identical SPMD program).

Per core on-device pipeline (activations kept feature-major [feature, token]):
  LN1 over full 2048 tokens (for K/V) and over the 1024 query tokens (for Q),
  QKV projections, causal attention (scores computed transposed [keys, q] with
  key-tile-outer loop; softmax without max-subtraction; per-tile multiplicative
  masks from host), Wo + residual, LN2, MLP (exact GELU) + residual.

All matmuls in bf16 with fp32 PSUM accumulation; LN stats, residuals and
softmax normalization in fp32.
"""

import numpy as np

N_CORES = 8
B, S, H, D = 4, 2048, 12, 64
HID = 768
QL = 1024          # query rows per core
KT = HID // 128    # 6 feature k-tiles
TT = S // 128      # 16 token tiles
MH = 4 * HID // 128  # 24 tiles of the MLP hidden dim
EPS = 1e-5

_CACHE = {}


def _build_program():
    from contextlib import ExitStack
    import concourse.bass as bass
    import concourse.tile as tile
    from concourse import bacc, mybir

    F32 = mybir.dt.float32
    BF16 = mybir.dt.bfloat16
    Alu = mybir.AluOpType
    Act = mybir.ActivationFunctionType

    nc = bacc.Bacc("TRN2", target_bir_lowering=False, debug=False,
                   enable_asserts=False, num_devices=N_CORES)

    def din(name, shape, dt):
        return nc.dram_tensor(name, shape, dt, kind="ExternalInput").ap()

    # --- per-core inputs ---
    xbT = din("xbT", [HID, S], BF16)          # x[b].T, bf16
    xbTq = din("xbTq", [HID, QL], BF16)       # gathered query cols, bf16
    xTq = din("xTq", [HID, QL], F32)          # gathered query cols, f32 (residual)
    masks = din("masks", [TT, 128, 128], BF16)
    Wq = din("Wq", [HID, HID], BF16)
    Wk = din("Wk", [HID, HID], BF16)
    Wv = din("Wv", [HID, HID], BF16)
    Wo = din("Wo", [HID, HID], BF16)
    W1 = din("W1", [HID, 4 * HID], BF16)
    W2 = din("W2", [4 * HID, HID], BF16)
    ln1w = din("ln1w", [HID], F32)
    ln1b = din("ln1b", [HID], F32)
    ln2w = din("ln2w", [HID], F32)
    ln2b = din("ln2b", [HID], F32)
    bqs = din("bqs", [HID], F32)              # bq / sqrt(D)
    bk = din("bk", [HID], F32)
    bv = din("bv", [HID], F32)
    bo = din("bo", [HID], F32)
    b1 = din("b1", [4 * HID], F32)
    b2 = din("b2", [HID], F32)

    y = nc.dram_tensor("y", [HID, QL], F32, kind="ExternalOutput").ap()

    # --- DRAM scratch ---
    st_dram = nc.dram_tensor("st_scratch", [16, S], F32).ap()
    rec_dram = nc.dram_tensor("rec_scratch", [H, QL], F32).ap()

    def bcast(src_elem_ap, parts, n):
        """AP reading n consecutive DRAM elems broadcast across `parts` partitions."""
        return bass.AP(tensor=src_elem_ap.tensor, offset=src_elem_ap.offset,
                       ap=[[0, parts], [1, n]])

    with tile.TileContext(nc) as tc, ExitStack() as ctx:
        sb = ctx.enter_context(tc.tile_pool(name="sb", bufs=1))
        ps = ctx.enter_context(tc.tile_pool(name="ps", bufs=1, space="PSUM"))

        # ---------- constant / parameter tiles ----------
        ones_bf = sb.tile([128, 1], BF16, tag="ones")
        nc.vector.memset(ones_bf, 1.0)
        # packed params: [128, 80] f32
        # cols 0:6 ln1w, 6:12 ln1b, 12:18 ln2w, 18:24 ln2b, 24:30 bqs, 30:36 bk,
        # 36:42 bo, 42:48 b2, 48:72 b1, 72 eps
        par = sb.tile([128, 80], F32, tag="par")

        def load_cols(dst0, src, n):  # [n*128] -> [128, n] cols of par
            nc.gpsimd.dma_start(
                out=par[:, dst0:dst0 + n],
                in_=bass.AP(tensor=src.tensor, offset=src.offset,
                            ap=[[1, 128], [128, n]]))

        load_cols(0, ln1w, KT)
        load_cols(6, ln1b, KT)
        load_cols(12, ln2w, KT)
        load_cols(18, ln2b, KT)
        load_cols(24, bqs, KT)
        load_cols(30, bk, KT)
        load_cols(36, bo, KT)
        load_cols(42, b2, KT)
        load_cols(48, b1, MH)
        nc.vector.memset(par[:, 72:73], EPS)
        eps_t = par[:, 72:73]
        bv_b = sb.tile([128, HID], F32, tag="bv_b")
        nc.gpsimd.dma_start(out=bv_b, in_=bcast(bv[0], 128, HID))

        masks_sb = sb.tile([128, TT, 128], BF16, tag="masks")
        nc.gpsimd.dma_start(out=masks_sb, in_=masks.rearrange("t p q -> p t q"))

        # ---------- LN helper (feature-major), chunk-pipelined ----------
        def emit_ln(N, x_bf, w0, b0, out_bf, row0):
            nchunks = N // 512
            for c in range(nchunks):
                off = 512 * c
                cs = slice(off, off + 512)
                s_ps = ps.tile([1, 512], F32, tag="s", bufs=8, name=f"sps{row0}_{c}")
                q_ps = ps.tile([1, 512], F32, tag="s", bufs=8, name=f"qps{row0}_{c}")
                for kt in range(KT):
                    nc.tensor.matmul(s_ps, ones_bf, x_bf[:, kt, cs],
                                     start=(kt == 0), stop=(kt == KT - 1))
                for kt in range(KT):
                    sqc = sb.tile([128, 512], BF16, tag="sq", bufs=2,
                                  name=f"sq{row0}_{c}_{kt}")
                    nc.vector.tensor_mul(sqc, x_bf[:, kt, cs], x_bf[:, kt, cs])
                    nc.tensor.matmul(q_ps, ones_bf, sqc,
                                     start=(kt == 0), stop=(kt == KT - 1))
                s_sb = sb.tile([1, 512], F32, tag="sts", bufs=2, name=f"ssb{row0}_{c}")
                q_sb = sb.tile([1, 512], F32, tag="sts", bufs=2, name=f"qsb{row0}_{c}")
                nc.scalar.copy(s_sb, s_ps)
                nc.scalar.copy(q_sb, q_ps)
                nc.gpsimd.dma_start(out=st_dram[row0, off:off + 512], in_=s_sb)
                nc.gpsimd.dma_start(out=st_dram[row0 + 1, off:off + 512], in_=q_sb)

                def resh(row):
                    base = st_dram[row, off]
                    return bass.AP(tensor=base.tensor, offset=base.offset,
                                   ap=[[1, 128], [128, 4]])
                s_r = sb.tile([128, 4], F32, tag="str", bufs=8, name=f"sr{row0}_{c}")
                q_r = sb.tile([128, 4], F32, tag="str", bufs=8, name=f"qr{row0}_{c}")
                nc.gpsimd.dma_start(out=s_r, in_=resh(row0))
                nc.gpsimd.dma_start(out=q_r, in_=resh(row0 + 1))
                mu = sb.tile([128, 4], F32, tag="str", bufs=8, name=f"mu{row0}_{c}")
                e2 = sb.tile([128, 4], F32, tag="str", bufs=8, name=f"e2{row0}_{c}")
                nc.scalar.mul(mu, s_r, 1.0 / HID)
                nc.scalar.mul(e2, q_r, 1.0 / HID)
                var = sb.tile([128, 4], F32, tag="str", bufs=8, name=f"var{row0}_{c}")
                nc.vector.tensor_mul(var, mu, mu)
                nc.vector.tensor_sub(var, e2, var)
                sd = sb.tile([128, 4], F32, tag="str", bufs=8, name=f"sd{row0}_{c}")
                nc.scalar.activation(sd, var, Act.Sqrt, bias=eps_t, scale=1.0)
                a_r = sb.tile([128, 4], F32, tag="str", bufs=8, name=f"ar{row0}_{c}")
                nc.vector.reciprocal(a_r, sd)
                c_r = sb.tile([128, 4], F32, tag="str", bufs=8, name=f"cr{row0}_{c}")
                nc.vector.tensor_mul(c_r, mu, a_r)
                nc.scalar.mul(c_r, c_r, -1.0)
                nc.gpsimd.dma_start(out=resh(row0 + 2), in_=a_r)
                nc.gpsimd.dma_start(out=resh(row0 + 3), in_=c_r)
                a_b = sb.tile([128, 512], F32, tag="ab", bufs=2, name=f"ab{row0}_{c}")
                c_b = sb.tile([128, 512], F32, tag="ab", bufs=2, name=f"cb{row0}_{c}")
                nc.gpsimd.dma_start(out=a_b, in_=bcast(st_dram[row0 + 2, off], 128, 512))
                nc.gpsimd.dma_start(out=c_b, in_=bcast(st_dram[row0 + 3, off], 128, 512))
                for kt in range(KT):
                    t0 = sb.tile([128, 512], F32, tag="t0", bufs=2,
                                 name=f"t0{row0}_{c}_{kt}")
                    nc.vector.tensor_mul(t0, x_bf[:, kt, cs], a_b)
                    nc.vector.tensor_add(t0, t0, c_b)
                    nc.vector.tensor_scalar(out_bf[:, kt, cs], t0,
                                            par[:, w0 + kt:w0 + kt + 1],
                                            par[:, b0 + kt:b0 + kt + 1],
                                            Alu.mult, Alu.add)

        # ---------- Phase 1: LN1 (full) and LN1q ----------
        xbT_sb = sb.tile([128, KT, S], BF16, tag="fatA")
        for c in range(S // 512):
            nc.sync.dma_start(
                out=xbT_sb[:, :, 512 * c:512 * c + 512],
                in_=xbT.rearrange("(k p) s -> p k s", p=128)[:, :, 512 * c:512 * c + 512])
        xbTq_sb = sb.tile([128, KT, QL], BF16, tag="medB")
        for c in range(QL // 512):
            nc.sync.dma_start(
                out=xbTq_sb[:, :, 512 * c:512 * c + 512],
                in_=xbTq.rearrange("(k p) s -> p k s", p=128)[:, :, 512 * c:512 * c + 512])

        ln_bf = sb.tile([128, KT, S], BF16, tag="fatB")
        lnq_bf = sb.tile([128, KT, QL], BF16, tag="medA")
        emit_ln(S, xbT_sb, 0, 6, ln_bf, 0)
        emit_ln(QL, xbTq_sb, 0, 6, lnq_bf, 4)

        # ---------- Phase 2: QKV projections ----------
        K_sb = sb.tile([128, KT, S], BF16, tag="fatC")
        for n in range(S // 512):
            cs = slice(512 * n, 512 * n + 512)
            for mo in range(KT):
                wkt = sb.tile([128, KT, 128], BF16, tag="wk6", bufs=3,
                              name=f"wk{n}_{mo}")
                nc.sync.dma_start(
                    out=wkt,
                    in_=Wk.rearrange("(k p) m -> p k m", p=128)[:, :, 128 * mo:128 * mo + 128])
                pst = ps.tile([128, 512], F32, tag="s", bufs=8, name=f"kps{mo}_{n}")
                for kt in range(KT):
                    nc.tensor.matmul(pst, wkt[:, kt, :], ln_bf[:, kt, cs],
                                     start=(kt == 0), stop=(kt == KT - 1))
                nc.vector.tensor_scalar(K_sb[:, mo, cs], pst,
                                        par[:, 30 + mo:31 + mo], None, Alu.add)

        Q_sb = sb.tile([128, KT, QL], BF16, tag="qsb")
        for n in range(QL // 512):
            cs = slice(512 * n, 512 * n + 512)
            for mo in range(KT):
                wqt = sb.tile([128, KT, 128], BF16, tag="wk6", bufs=3,
                              name=f"wq{n}_{mo}")
                nc.sync.dma_start(
                    out=wqt,
                    in_=Wq.rearrange("(k p) m -> p k m", p=128)[:, :, 128 * mo:128 * mo + 128])
                pst = ps.tile([128, 512], F32, tag="s", bufs=8, name=f"qps2{mo}_{n}")
                for kt in range(KT):
                    nc.tensor.matmul(pst, wqt[:, kt, :], lnq_bf[:, kt, cs],
                                     start=(kt == 0), stop=(kt == KT - 1))
                # (q + bq)/sqrt(D): psum*0.125 + bqs (bqs pre-scaled on host)
                nc.vector.tensor_scalar(Q_sb[:, mo, cs], pst,
                                        0.125, par[:, 24 + mo:25 + mo],
                                        Alu.mult, Alu.add)

        # V: token-major [128, tt, 12*65], head h at cols 65h..65h+63, ones at 65h+64
        V_sb = sb.tile([128, TT, H * 65], BF16, tag="vsb")
        for fc in range(2):  # fout chunks of 384 = 6 heads
            wvt = sb.tile([128, KT, 384], BF16, tag="wv", bufs=1, name=f"wv{fc}")
            nc.sync.dma_start(
                out=wvt,
                in_=Wv.rearrange("(k p) m -> p k m", p=128)[:, :, 384 * fc:384 * fc + 384])
            for tt in range(TT):
                pst = ps.tile([128, 384], F32, tag="s", bufs=8, name=f"vps{tt}_{fc}")
                for kt in range(KT):
                    nc.tensor.matmul(pst, ln_bf[:, kt, 128 * tt:128 * tt + 128],
                                     wvt[:, kt, :],
                                     start=(kt == 0), stop=(kt == KT - 1))
                vdst = V_sb[:, tt, :].rearrange("p (h e) -> p h e", e=65)[:, 6 * fc:6 * fc + 6, 0:64]
                nc.vector.tensor_tensor(vdst, pst.rearrange("p (h d) -> p h d", d=64),
                                        bv_b[:, 384 * fc:384 * fc + 384].rearrange(
                                            "p (h d) -> p h d", d=64),
                                        Alu.add)
        for tt in range(TT):
            nc.vector.memset(
                V_sb[:, tt, :].rearrange("p (h e) -> p h e", e=65)[:, :, 64:65], 1.0)

        # ---------- Phase 3: attention ----------
        attn_bf = sb.tile([128, KT, QL], BF16, tag="medA")
        for h in range(H):
            kt = h // 2
            p0 = 64 * (h % 2)
            rows = slice(p0, p0 + 64)
            O_a = ps.tile([65, 512], F32, tag="s", bufs=8, name=f"oa{h}")
            O_b = ps.tile([65, 512], F32, tag="s", bufs=8, name=f"ob{h}")

            def av(tile_t, e, q0):
                if q0 < 512:
                    nc.tensor.matmul(O_a[:, q0:512],
                                     V_sb[:, tile_t, 65 * h:65 * h + 65],
                                     e[:, 0:512 - q0],
                                     start=(tile_t == 0), stop=(tile_t == TT - 1))
                nc.tensor.matmul(O_b[:, max(q0, 512) - 512:512],
                                 V_sb[:, tile_t, 65 * h:65 * h + 65],
                                 e[:, max(q0, 512) - q0:QL - q0],
                                 start=(tile_t == 0), stop=(tile_t == TT - 1))

            prev = None  # (expS tile, q0, t)
            for t in range(TT):
                q0 = 128 * (t // 2)
                span = QL - q0
                expS = sb.tile([128, span], BF16, tag="expS", bufs=3,
                               name=f"es{h}_{t}")
                chunks = ([(q0, 512), (512, QL)] if q0 < 512 else [(q0, QL)])
                for (cs, ce) in chunks:
                    S_ps = ps.tile([128, ce - cs], F32, tag="s", bufs=8,
                                   name=f"scr{h}_{t}_{cs}")
                    nc.tensor.matmul(S_ps, K_sb[rows, kt, 128 * t:128 * t + 128],
                                     Q_sb[rows, kt, cs:ce], start=True, stop=True)
                    nc.scalar.activation(expS[:, cs - q0:ce - q0], S_ps, Act.Exp)
                nc.gpsimd.tensor_mul(expS[:, 0:128], expS[:, 0:128],
                                     masks_sb[:, t, :])
                if prev is not None:
                    pe, pq0, pt = prev
                    av(pt, pe, pq0)
                prev = (expS, q0, t)
            pe, pq0, pt = prev
            av(pt, pe, pq0)
            rec = sb.tile([65, QL], F32, tag="rec", bufs=1, name=f"rec{h}")
            nc.vector.reciprocal(rec[64:65, 0:512], O_a[64:65, :])
            nc.vector.reciprocal(rec[64:65, 512:QL], O_b[64:65, :])
            nc.gpsimd.dma_start(out=rec_dram[h, :], in_=rec[64:65, :])
            nc.gpsimd.dma_start(out=rec[0:64, :], in_=bcast(rec_dram[h, 0], 64, QL))
            if h % 2 == 0:
                nc.vector.tensor_mul(attn_bf[0:64, kt, 0:512], O_a[0:64, :],
                                     rec[0:64, 0:512])
                nc.vector.tensor_mul(attn_bf[0:64, kt, 512:QL], O_b[0:64, :],
                                     rec[0:64, 512:QL])
            else:
                stg = sb.tile([64, QL], BF16, tag="stg", bufs=1, name=f"stg{h}")
                nc.vector.tensor_mul(stg[:, 0:512], O_a[0:64, :], rec[0:64, 0:512])
                nc.vector.tensor_mul(stg[:, 512:QL], O_b[0:64, :], rec[0:64, 512:QL])
                nc.sync.dma_start(out=attn_bf[64:128, kt, :], in_=stg)

        # ---------- Phase 4: Wo + residual + LN2 ----------
        xTq_sb = sb.tile([128, KT, QL], F32, tag="fatA")
        nc.sync.dma_start(out=xTq_sb, in_=xTq.rearrange("(k p) s -> p k s", p=128))
        r_sb = sb.tile([128, KT, QL], F32, tag="fatB")
        rb_sb = sb.tile([128, KT, QL], BF16, tag="medB")
        for n in range(QL // 512):
            cs = slice(512 * n, 512 * n + 512)
            for mo in range(KT):
                wot = sb.tile([128, KT, 128], BF16, tag="wk6", bufs=3,
                              name=f"wo{n}_{mo}")
                nc.sync.dma_start(
                    out=wot,
                    in_=Wo.rearrange("(k p) m -> p k m", p=128)[:, :, 128 * mo:128 * mo + 128])
                pst = ps.tile([128, 512], F32, tag="s", bufs=8, name=f"ops2{mo}_{n}")
                for kt in range(KT):
                    nc.tensor.matmul(pst, wot[:, kt, :], attn_bf[:, kt, cs],
                                     start=(kt == 0), stop=(kt == KT - 1))
                nc.vector.scalar_tensor_tensor(r_sb[:, mo, cs], pst,
                                               par[:, 36 + mo:37 + mo],
                                               xTq_sb[:, mo, cs],
                                               Alu.add, Alu.add)
                nc.vector.tensor_copy(rb_sb[:, mo, cs], r_sb[:, mo, cs])
        ln2_bf = sb.tile([128, KT, QL], BF16, tag="medC")
        emit_ln(QL, rb_sb, 12, 18, ln2_bf, 8)

        # ---------- Phase 5: MLP (token-half sweep, g stays in SBUF) ----------
        y_sb = sb.tile([128, KT, QL], F32, tag="fatC")
        for n in range(QL // 512):
            cs = slice(512 * n, 512 * n + 512)
            g_half = sb.tile([128, MH, 512], BF16, tag="vsb", name=f"gh{n}")
            for mo in range(MH):
                w1t = sb.tile([128, KT, 128], BF16, tag="wk6", bufs=3,
                              name=f"w1{n}_{mo}")
                nc.sync.dma_start(
                    out=w1t,
                    in_=W1.rearrange("(k p) m -> p k m", p=128)[:, :, 128 * mo:128 * mo + 128])
                pst = ps.tile([128, 512], F32, tag="s", bufs=8, name=f"h1ps{n}_{mo}")
                for kt in range(KT):
                    nc.tensor.matmul(pst, w1t[:, kt, :], ln2_bf[:, kt, cs],
                                     start=(kt == 0), stop=(kt == KT - 1))
                nc.scalar.activation(g_half[:, mo, :], pst, Act.Gelu,
                                     bias=par[:, 48 + mo:49 + mo], scale=1.0)
            psts = [ps.tile([128, 512], F32, tag="s", bufs=8, name=f"yps{n}_{i}")
                    for i in range(KT)]
            for kp in range(MH // 2):
                w2t = sb.tile([128, 2, HID], BF16, tag="w2", bufs=2, name=f"w2{n}_{kp}")
                nc.sync.dma_start(
                    out=w2t,
                    in_=W2.rearrange("(a p) m -> p a m", p=128)[:, 2 * kp:2 * kp + 2, :])
                for j in range(2):
                    k2 = 2 * kp + j
                    for mo in range(KT):
                        nc.tensor.matmul(psts[mo], w2t[:, j, 128 * mo:128 * mo + 128],
                                         g_half[:, k2, :],
                                         start=(k2 == 0), stop=(k2 == MH - 1))
            for mo in range(KT):
                nc.vector.scalar_tensor_tensor(y_sb[:, mo, cs], psts[mo],
                                               par[:, 42 + mo:43 + mo],
                                               r_sb[:, mo, cs],
                                               Alu.add, Alu.add)
                nc.sync.dma_start(out=y[128 * mo:128 * mo + 128, cs],
                                  in_=y_sb[:, mo, cs])

    nc.compile()
    return nc


def _get_program():
    if "nc" not in _CACHE:
        _CACHE["nc"] = _build_program()
    return _CACHE["nc"]


def _prep_in_maps(inputs):
    import ml_dtypes
    bf = ml_dtypes.bfloat16
    f32 = np.float32

    x = np.ascontiguousarray(np.asarray(inputs["x"], dtype=f32))
    shared = {
        "Wq": np.ascontiguousarray(np.asarray(inputs["Wq"], f32).astype(bf)),
        "Wk": np.ascontiguousarray(np.asarray(inputs["Wk"], f32).astype(bf)),
        "Wv": np.ascontiguousarray(np.asarray(inputs["Wv"], f32).astype(bf)),
        "Wo": np.ascontiguousarray(np.asarray(inputs["Wo"], f32).astype(bf)),
        "W1": np.ascontiguousarray(np.asarray(inputs["W1"], f32).astype(bf)),
        "W2": np.ascontiguousarray(np.asarray(inputs["W2"], f32).astype(bf)),
        "ln1w": np.asarray(inputs["ln1_w"], f32),
        "ln1b": np.asarray(inputs["ln1_b"], f32),
        "ln2w": np.asarray(inputs["ln2_w"], f32),
        "ln2b": np.asarray(inputs["ln2_b"], f32),
        "bqs": (np.asarray(inputs["bq"], f32) / np.float32(np.sqrt(D))),
        "bk": np.asarray(inputs["bk"], f32),
        "bv": np.asarray(inputs["bv"], f32),
        "bo": np.asarray(inputs["bo"], f32),
        "b1": np.asarray(inputs["b1"], f32),
        "b2": np.asarray(inputs["b2"], f32),
    }

    in_maps = []
    qcols_all = []
    for c in range(N_CORES):
        b, p = c // 2, c % 2
        xT = np.ascontiguousarray(x[b].T)                        # [768, 2048]
        qcols = np.concatenate(
            [np.arange(128 * (2 * j + p), 128 * (2 * j + p) + 128) for j in range(8)])
        qcols_all.append(qcols)
        xTq = np.ascontiguousarray(xT[:, qcols])
        m = np.zeros((TT, 128, 128), np.float32)
        for t in range(TT):
            g = 2 * (t // 2) + p
            kk = 128 * t + np.arange(128)[:, None]
            qq = 128 * g + np.arange(128)[None, :]
            m[t] = (kk <= qq).astype(np.float32)
        im = dict(shared)
        im["xbT"] = xT.astype(bf)
        im["xbTq"] = xTq.astype(bf)
        im["xTq"] = xTq
        im["masks"] = m.astype(bf)
        in_maps.append(im)
    return in_maps, qcols_all


def kernel(**inputs):
    import sys, types
    # register the NTFF profile hook shim (harmless if profiling unused)
    if "antenv.axon_hooks" not in sys.modules:
        try:
            sys.path.insert(0, "/root/.axon_site")
            from trn_agent_boot.trn_boot import _ntff_profile_via_ctypes
            hook = _ntff_profile_via_ctypes("/opt/axon/libaxon_pjrt.so")
            mod = types.ModuleType("antenv.axon_hooks")
            mod.get_axon_ntff_profile_hook = lambda: hook
            mod.set_axon_ntff_profile_hook = lambda h: None
            import antenv  # noqa: F401
            sys.modules["antenv.axon_hooks"] = mod
        except Exception:
            pass

    from concourse.bass_utils import run_bass_kernel_spmd

    nc = _get_program()
    in_maps, qcols_all = _prep_in_maps(inputs)
    res = run_bass_kernel_spmd(nc, in_maps, core_ids=list(range(N_CORES)))
    out = np.zeros((B, S, HID), np.float32)
    for c in range(N_CORES):
        out[c // 2, qcols_all[c], :] = res.results[c]["y"].T
    return out
